# revision 1
# baseline (speedup 1.0000x reference)
"""Trainium2 Bass kernel for nn_CurvatureReg:
mean(tv_min(curvature(gauss(pred))) * dilate_mask(pred)).

Sharding: 16 logical shards = 2 batches x 4 D-chunks x 2 H-halves; two shards
run sequentially per core sharing one SBUF tile set. Per-shard layout:
partitions = H (94 = 80 owned + 7 halo), free = (D_local 54, W).
D/W stencils: shifted-AP DVE ops. H 9-tap blur: banded f32 matmul on PE.
H +/-1 stencils: partition-shifted SBUF->SBUF DMA + DVE ops. Host sums
per-partition partials over owned rows only.
"""
import os
import numpy as np
import ml_dtypes

import concourse.bacc as bacc
import concourse.bass as bass
import concourse.mybir as mybir
import concourse.tile as tile
from concourse.bass_utils import run_bass_kernel_spmd

dt = mybir.dt
F32 = dt.float32
BF16 = dt.bfloat16
ALU = mybir.AluOpType
ACTF = mybir.ActivationFunctionType

_NCORES = int(os.environ.get("KCORES", "8"))
_REPEAT = int(os.environ.get("KREPEAT", "1"))

NB, DIM = 2, 160
CHUNK = 40                # owned D planes per core
HALO = 7
DEXT = CHUNK + 2 * HALO   # 54
WPAD = DIM + 8
HOWN = 80                 # owned H rows per shard
HEXT = HOWN + 2 * HALO    # 94 partitions
THRES = 0.1
EPS = 1e-6
CH = 3                    # D-rows per PE chunk (3*156*4B < 2KB PSUM bank)


class Field:
    def __init__(self, t, d_base, w_base):
        self.t = t
        self.d0 = d_base
        self.w0 = w_base

    def ap(self, d_lo, d_hi, w_lo, w_hi, p0=0, p1=HEXT):
        return self.t[p0:p1, d_lo - self.d0:d_hi - self.d0,
                      w_lo - self.w0:w_hi - self.w0]


def _build(kd, kw):
    nc = bacc.Bacc(None, target_bir_lowering=False, debug=False)

    ins = {}
    for h in (0, 1):
        ins[f"slab{h}"] = nc.dram_tensor(f"slab{h}", [HEXT, DEXT, WPAD], F32,
                                         kind="ExternalInput").ap()
        ins[f"bb{h}"] = nc.dram_tensor(f"bb{h}", [HEXT, HEXT], F32,
                                       kind="ExternalInput").ap()
    dmask_d = nc.dram_tensor("dmask", [HEXT, 16], BF16, kind="ExternalInput").ap()
    part_d = [nc.dram_tensor(f"partial{h}", [HEXT, 1], F32,
                             kind="ExternalOutput").ap() for h in (0, 1)]

    P = HEXT
    with tile.TileContext(nc) as tc:
        with tc.tile_pool(name="big", bufs=1) as pool, \
             tc.tile_pool(name="ps", bufs=2, space="PSUM") as psp:

            def mk(rows, cols, dtp, name):
                return pool.tile([P, rows, cols], dtp, tag=name, name=name)

            S1 = mk(DEXT, WPAD, F32, "S1")   # pred (w base -4)
            S2 = mk(DEXT, 156, F32, "S2")    # A -> phi -> tmp -> ddx/k
            S3 = mk(46, 156, F32, "S3")      # Bf -> Gy/n'y -> |dH|
            S4 = mk(44, 154, F32, "S4")      # (bf16 scratch) -> Gx/n'x -> dW/score
            S5 = mk(44, 154, F32, "S5")      # Gz/n'z -> ddy-align -> dD
            S6 = mk(44, 154, F32, "S6")      # shiftT -> s/t/rec -> ddz -> shiftT
            S7 = mk(42, 152, BF16, "S7")     # b -> boxH -> maskf
            S8 = mk(42, 152, BF16, "S8")     # boxW -> boxD
            bblur_t = pool.tile([P, HEXT], F32, tag="bblur", name="bblur")
            dmask_t = pool.tile([P, 16], BF16, tag="dmaskt", name="dmaskt")
            pacc = [pool.tile([P, 1], F32, tag=f"pacc{h}", name=f"pacc{h}")
                    for h in (0, 1)]
            pdum = pool.tile([P, 1], F32, tag="pdum", name="pdum")

            pred = Field(S1, 0, -4)
            A = Field(S2, 0, 2)
            phi = Field(S2, 0, 2)
            Bf = Field(S3, 4, 2)
            Gy = Field(S3, 5, 3)
            absdH = Field(S3, 7, 5)
            Gx = Field(S4, 5, 3)
            dW = Field(S4, 7, 5)
            Gz = Field(S5, 5, 3)
            ddyA = Field(S5, 6, 4)
            dD = Field(S5, 7, 5)
            srec = Field(S6, 5, 3)
            ddz = Field(S6, 6, 4)
            tmp = Field(S2, 5, 3)
            ddx = Field(S2, 6, 4)
            b = Field(S7, 6, 4)
            boxH = Field(S7, 6, 4)
            maskf = Field(S7, 7, 5)
            boxW = Field(S8, 6, 4)
            boxD = Field(S8, 7, 5)
            # bf16 view of S4 for the boxH shift-back scratch
            S4b = S4.bitcast(BF16)
            bx3 = Field(S4b, 6, 4)

            nc.vector.memset(pdum, 0.0)

            def tt(dst, a, b_, op, region, p1=HEXT):
                d0, d1, w0, w1 = region
                nc.vector.tensor_tensor(dst.ap(d0, d1, w0, w1, 0, p1),
                                        a.ap(d0, d1, w0, w1, 0, p1),
                                        b_.ap(d0, d1, w0, w1, 0, p1), op)

            def tt_shift(dst, a, b_, op, region, da=0, wa=0, db=0, wb=0):
                d0, d1, w0, w1 = region
                nc.vector.tensor_tensor(
                    dst.ap(d0, d1, w0, w1),
                    a.ap(d0 + da, d1 + da, w0 + wa, w1 + wa),
                    b_.ap(d0 + db, d1 + db, w0 + wb, w1 + wb), op)

            def blur_free(dst, src, taps, region, axis):
                d0, d1, w0, w1 = region
                dst_ap = dst.ap(d0, d1, w0, w1)
                for i, k in enumerate(range(-4, 5)):
                    da, wa = (k, 0) if axis == "d" else (0, k)
                    src_ap = src.ap(d0 + da, d1 + da, w0 + wa, w1 + wa)
                    if i == 0:
                        nc.scalar.mul(dst_ap, src_ap, float(taps[i]))
                    else:
                        nc.vector.scalar_tensor_tensor(
                            dst_ap, src_ap, float(taps[i]), dst_ap,
                            ALU.mult, ALU.add)

            def hdiff(src, T, out, region, op):
                """out(h) <- src(h+1) op-combine src(h-1) via 2 shifted DMAs.

                T: scratch Field (same region geometry). op: subtract or add.
                out rows [1,93) valid; row 0/93 garbage (unowned, host-sliced).
                """
                d0, d1, w0, w1 = region
                nc.sync.dma_start(out=T.ap(d0, d1, w0, w1, 0, 92),
                                  in_=src.ap(d0, d1, w0, w1, 2, 94))
                nc.vector.tensor_tensor(T.ap(d0, d1, w0, w1, 0, 92),
                                        T.ap(d0, d1, w0, w1, 0, 92),
                                        src.ap(d0, d1, w0, w1, 0, 92), op)
                nc.sync.dma_start(out=out.ap(d0, d1, w0, w1, 1, 93),
                                  in_=T.ap(d0, d1, w0, w1, 0, 92))

            def pe_blurh(dst, src, region):
                """9-tap H blur via banded matmul, 4-chunk grouped evacuation."""
                d0, d1, w0, w1 = region
                wn = w1 - w0
                chunks = []
                c0 = d0
                while c0 < d1:
                    chunks.append((c0, min(c0 + CH, d1)))
                    c0 = min(c0 + CH, d1)
                for g in range(0, len(chunks), 4):
                    grp = chunks[g:g + 4]
                    ps = psp.tile([P, 4, 512], F32, tag="ps", name="ps")
                    for ci, (a0, a1) in enumerate(grp):
                        nc.tensor.matmul(ps[:, ci:ci + 1, 0:(a1 - a0) * wn], bblur_t,
                                         src.ap(a0, a1, w0, w1),
                                         start=True, stop=True)
                    full = [c for c in grp if c[1] - c[0] == CH]
                    if full:
                        nfull = len(full)
                        pin = ps[0:P, 0:nfull, 0:CH * wn]
                        dbase = dst.ap(full[0][0], full[-1][1], w0, w1)
                        dout = bass.AP(tensor=dbase.tensor, offset=dbase.offset,
                                       ap=[dbase.ap[0], [CH * wn, nfull],
                                           [1, CH * wn]])
                        nc.scalar.copy(dout, pin)
                    for (a0, a1) in grp[len(full):]:
                        nc.scalar.copy(dst.ap(a0, a1, w0, w1),
                                       ps[:, len(full):len(full) + 1,
                                          0:(a1 - a0) * wn])

            def shard(h):
                nc.sync.dma_start(out=S1, in_=ins[f"slab{h}"])
                nc.sync.dma_start(out=bblur_t, in_=ins[f"bb{h}"])
                if h == 0:
                    nc.sync.dma_start(out=dmask_t, in_=dmask_d)

                # ---- mask pipeline (bf16) ----
                nc.vector.tensor_scalar(b.ap(6, 48, 4, 156),
                                        pred.ap(6, 48, 4, 156),
                                        THRES, None, ALU.is_ge)
                MB = (6, 48, 5, 155)
                tt_shift(boxW, b, b, ALU.add, MB, wa=-1, wb=1)
                tt(boxW, boxW, b, ALU.add, MB)
                # boxH(h) = boxW(h-1)+boxW(h)+boxW(h+1) via shifted DMAs
                d0, d1, w0, w1 = MB
                nc.sync.dma_start(out=boxH.ap(d0, d1, w0, w1, 0, 92),
                                  in_=boxW.ap(d0, d1, w0, w1, 2, 94))
                nc.vector.tensor_tensor(boxH.ap(d0, d1, w0, w1, 0, 92),
                                        boxH.ap(d0, d1, w0, w1, 0, 92),
                                        boxW.ap(d0, d1, w0, w1, 0, 92), ALU.add)
                nc.sync.dma_start(out=bx3.ap(d0, d1, w0, w1, 1, 93),
                                  in_=boxH.ap(d0, d1, w0, w1, 0, 92))
                tt(boxH, bx3, boxW, ALU.add, MB, p1=93)
                tt_shift(boxD, boxH, boxH, ALU.add, (7, 47, 5, 155), da=-1, db=1)
                tt(boxD, boxD, boxH, ALU.add, (7, 47, 5, 155))
                nc.vector.tensor_scalar(maskf.ap(7, 47, 5, 155),
                                        boxD.ap(7, 47, 5, 155),
                                        0.5, None, ALU.is_ge)
                # zero global-D borders (cols 0-9) and H borders (col 10+h)
                base = S7[:, 0:5, 0:150]
                m_ap = bass.AP(tensor=base.tensor, offset=base.offset,
                               ap=[base.ap[0], [35 * 152, 2], [152, 5], [1, 150]])
                dbase = dmask_t[:, 0:1]
                dm_ap = bass.AP(tensor=dbase.tensor, offset=dbase.offset,
                               ap=[dbase.ap[0], [5, 2], [1, 5], [0, 150]])
                nc.vector.tensor_tensor(m_ap, m_ap, dm_ap, ALU.mult)
                hbase = dmask_t[:, 10 + h:11 + h]
                hm_ap = bass.AP(tensor=hbase.tensor, offset=hbase.offset,
                                ap=[hbase.ap[0], [0, 40], [0, 150]])
                mf = maskf.ap(7, 47, 5, 155)
                nc.vector.tensor_tensor(mf, mf, hm_ap, ALU.mult)

                # ---- separable Gaussian ----
                blur_free(A, pred, kw, (0, 54, 2, 158), "w")
                blur_free(Bf, A, kd, (4, 50, 2, 158), "d")
                pe_blurh(phi, Bf, (4, 50, 2, 158))

                # ---- gradients ----
                R = (5, 49, 3, 157)
                tt_shift(Gx, phi, phi, ALU.subtract, R, da=1, db=-1)
                hdiff(phi, srec, Gy, R, ALU.subtract)
                tt_shift(Gz, phi, phi, ALU.subtract, R, wa=1, wb=-1)

                nc.scalar.square(srec.ap(*R), Gx.ap(*R))
                nc.scalar.square(tmp.ap(*R), Gy.ap(*R))
                tt(srec, srec, tmp, ALU.add, R)
                nc.scalar.square(tmp.ap(*R), Gz.ap(*R))
                tt(srec, srec, tmp, ALU.add, R)
                nc.scalar.activation(srec.ap(*R), srec.ap(*R),
                                     ACTF.Sqrt, bias=0.0, scale=4.0)
                nc.vector.tensor_scalar(srec.ap(*R), srec.ap(*R),
                                        4.0 * EPS, None, ALU.add)
                nc.vector.reciprocal(srec.ap(*R), srec.ap(*R))
                tt(Gx, Gx, srec, ALU.mult, R)
                tt(Gy, Gy, srec, ALU.mult, R)
                tt(Gz, Gz, srec, ALU.mult, R)

                # ---- curvature k = div n' ----
                K = (6, 48, 4, 156)
                tt_shift(ddx, Gx, Gx, ALU.subtract, K, da=1, db=-1)
                tt_shift(ddz, Gz, Gz, ALU.subtract, K, wa=1, wb=-1)
                tt(ddx, ddx, ddz, ALU.add, K)
                hdiff(Gy, ddz, ddyA, K, ALU.subtract)   # ddz slot as scratch
                tt(ddx, ddx, ddyA, ALU.add, K, p1=93)
                k = ddx

                # ---- tv-min ----
                V = (7, 47, 5, 155)
                tt_shift(dW, k, k, ALU.subtract, V, wa=1, wb=-1)
                nc.scalar.activation(dW.ap(*V), dW.ap(*V), ACTF.Abs)
                hdiff(k, srec, absdH, V, ALU.subtract)  # srec slot as scratch
                nc.scalar.activation(absdH.ap(*V), absdH.ap(*V), ACTF.Abs)
                tt(dW, dW, absdH, ALU.min, V)
                tt_shift(dD, k, k, ALU.subtract, V, da=1, db=-1)
                nc.scalar.activation(dD.ap(*V), dD.ap(*V), ACTF.Abs)
                tt(dW, dW, dD, ALU.min, V)

                # ---- masked sum ----
                sc = dW.ap(*V)
                db_ = pdum[:, 0:1]
                dum_ap = bass.AP(tensor=db_.tensor, offset=db_.offset,
                                 ap=[db_.ap[0], [0, 40], [0, 150]])
                nc.vector.scalar_tensor_tensor(
                    dum_ap, sc, 1.0, maskf.ap(*V),
                    ALU.mult, ALU.mult, accum_out=pacc[h])
                nc.sync.dma_start(out=part_d[h], in_=pacc[h])

            for _ in range(_REPEAT):
                shard(0)
                shard(1)

    nc.compile()
    return nc


_PROG = None


def _taps_from_gk(gk):
    g = np.asarray(gk, dtype=np.float64).reshape(9, 9, 9)
    return g.sum(axis=(1, 2)), g.sum(axis=(0, 2)), g.sum(axis=(0, 1))


def _band_blur(kh):
    r = np.arange(DIM)
    diff = r[:, None] - r[None, :]
    bblur = np.zeros((DIM, DIM), np.float64)
    m = np.abs(diff) <= 4
    bblur[m] = kh[(diff + 4)[m]]
    return bblur.astype(np.float32)


def _slice_band(B, h0):
    out = np.zeros((HEXT, HEXT), B.dtype)
    lo, hi = max(h0, 0), min(h0 + HEXT, DIM)
    out[lo - h0:hi - h0, lo - h0:hi - h0] = B[lo:hi, lo:hi]
    return out


def _in_maps(pred, gk):
    pred = np.asarray(pred, dtype=np.float32)
    kd, kh, kw = _taps_from_gk(gk)
    bblur = _band_blur(kh)
    in_maps = []
    for core in range(8):
        n, c = divmod(core, 4)
        gd0 = c * CHUNK - HALO
        dlo, dhi = max(gd0, 0), min(gd0 + DEXT, DIM)
        m = {}
        dmask = np.ones((HEXT, 16), np.float32)
        if c == 0:
            dmask[:, 0:5] = 0.0
        if c == 3:
            dmask[:, 5:10] = 0.0
        for h in (0, 1):
            h0 = h * HOWN - HALO
            hlo, hhi = max(h0, 0), min(h0 + HEXT, DIM)
            slab = np.zeros((HEXT, DEXT, WPAD), np.float32)
            block = pred[n, 0, dlo:dhi, hlo:hhi, :]      # (d, h, w)
            slab[hlo - h0:hhi - h0, dlo - gd0:dhi - gd0, 4:4 + DIM] = \
                block.transpose(1, 0, 2)
            m[f"slab{h}"] = slab
            m[f"bb{h}"] = _slice_band(bblur, h0)
            hg = h0 + np.arange(HEXT)
            dmask[:, 10 + h] = ((hg >= 5) & (hg < DIM - 5)).astype(np.float32)
        m["dmask"] = dmask.astype(ml_dtypes.bfloat16)
        in_maps.append(m)
    return in_maps


def kernel(pred, gk):
    global _PROG
    gk = np.asarray(gk, dtype=np.float32)
    kd, kh, kw = _taps_from_gk(gk)

    key = (tuple(kd), tuple(kw))
    if _PROG is None or _PROG[1] != key:
        _PROG = (_build(kd, kw), key)
    nc = _PROG[0]

    in_maps = _in_maps(pred, gk)
    res = run_bass_kernel_spmd(nc, in_maps[:_NCORES], list(range(_NCORES))).results
    total = 0.0
    for core in range(_NCORES):
        for h in (0, 1):
            p = np.asarray(res[core][f"partial{h}"], dtype=np.float64)
            total += p[HALO:HALO + HOWN].sum()
    return np.float32(total / (NB * DIM * DIM * DIM))

